# revision 5
# baseline (speedup 1.0000x reference)
"""Cond-LSTM Trainium2 kernel (nn_Cond_LSTM): batch-sharded SPMD over 8 NeuronCores.

Reference computation:
    f   = wei_F @ tnsr_cond                              (F,)
    WH  = einsum('ghf,f,gfk->ghk', wei_U, f, wei_V)      (4,H,H)
    xp  = einsum('ghi,tbi->tghb', wei_WI, tnsr_input)    (T,4,H,B)
    LSTM scan over T with z_t = xp_t + WH @ h_{t-1}
    out = stack([hs, cs])                                (2,T+1,H,B)

Sharding: data-parallel over batch (B=64 -> 8 per core); weights replicated.
All arithmetic runs on-device (bf16 matmuls, fp32 accumulation/activations).
Host does only slicing / layout rearranges / gather.
"""

import numpy as np

T, B, I, H, F, C = 256, 64, 1024, 1024, 512, 512
NG = 4            # gates
N_CORES = 8
BL = B // N_CORES  # local batch = 8
TBLK = 8           # recurrence steps per x-transpose/xp block
NBLK = T // TBLK
KSUB = H // 128    # 8 k-tiles over hidden dim
ISUB = I // 128    # 8 k-tiles over input dim
MT = NG * H // 128  # 32 m-tiles over z rows
# gate order used on-device: [i, f, o, g]; reference order is [i, f, g, o]
GATE_PERM = [0, 1, 3, 2]

_CACHED = None


def _build_program():
    import concourse.bacc as bacc
    import concourse.mybir as mybir
    import concourse.tile as tile

    F32 = mybir.dt.float32
    BF = mybir.dt.bfloat16
    AF = mybir.ActivationFunctionType

    nc = bacc.Bacc("TRN2", target_bir_lowering=False, debug=False,
                   num_devices=N_CORES)

    # ---- per-core inputs (host-rearranged layouts, all fp32) ----
    x_in = nc.dram_tensor("x", [T * BL, I], F32, kind="ExternalInput")
    h0_in = nc.dram_tensor("h0c", [128, KSUB * BL], F32, kind="ExternalInput")
    c0_in = nc.dram_tensor("c0c", [128, KSUB * BL], F32, kind="ExternalInput")
    wf_in = nc.dram_tensor("wFp", [128, F // 128, C], F32, kind="ExternalInput")
    cond_in = nc.dram_tensor("condb", [128, C], F32, kind="ExternalInput")
    v_in = nc.dram_tensor("Vp", [128, F // 128, NG, H], F32, kind="ExternalInput")
    ut_in = nc.dram_tensor("UTp", [128, F // 128, NG, H], F32, kind="ExternalInput")
    wit_in = nc.dram_tensor("WITp", [128, ISUB, NG * H], F32, kind="ExternalInput")
    out = nc.dram_tensor("out", [2, T + 1, H, BL], F32, kind="ExternalOutput")
    xbf_d = nc.dram_tensor("xbf_scratch", [T * BL, I], BF)

    FS = F // 128  # 4

    with tile.TileContext(nc) as tc:
        # ---------- resident weights ----------
        with tc.tile_pool(name="resident", bufs=1) as res:
            wht = res.tile([128, KSUB, NG * H], BF)   # lhsT for WH part (8 MB)
            wit = res.tile([128, ISUB, NG * H], BF)   # lhsT for xp part (8 MB)
            zero_sb = res.tile([128, 512], BF)
            nc.vector.memset(zero_sb[:], 0.0)

            # ---------- phase 0a: f = rowsum(wFp * cond) ----------
            with tc.tile_pool(name="p0a", bufs=1) as p0:
                wf_sb = p0.tile([128, FS, C], F32)
                nc.sync.dma_start(wf_sb[:], wf_in[:, :, :])
                cond_sb = p0.tile([128, C], F32)
                nc.sync.dma_start(cond_sb[:], cond_in[:, :])
                prod = p0.tile([128, FS, C], F32)
                nc.vector.tensor_tensor(
                    prod[:], wf_sb[:],
                    cond_sb[:, None, :].to_broadcast((128, FS, C)),
                    mybir.AluOpType.mult)
                fvec = p0.tile([128, FS], F32)
                nc.vector.reduce_sum(fvec[:], prod[:], axis=mybir.AxisListType.X)

                # ---------- phase 0b: WHT[g] = Vf[g]^T-contracted with UT[g] ----
                with tc.tile_pool(name="p0b", bufs=1) as pb, \
                     tc.tile_pool(name="p0ps", bufs=4, space="PSUM") as pps:
                    for g in range(NG):
                        utg_f = pb.tile([128, FS, H], F32, tag="utgf")
                        nc.sync.dma_start(utg_f[:], ut_in[:, :, g, :])
                        utg = pb.tile([128, FS, H], BF, tag="utg")
                        nc.scalar.copy(utg[:], utg_f[:])
                        vg_f = pb.tile([128, FS, H], F32, tag="vgf")
                        nc.sync.dma_start(vg_f[:], v_in[:, :, g, :])
                        vf = pb.tile([128, FS, H], BF, tag="vf")
                        nc.vector.tensor_tensor(
                            vf[:], vg_f[:],
                            fvec[:, :, None].to_broadcast((128, FS, H)),
                            mybir.AluOpType.mult)
                        for kh in range(KSUB):
                            for mch in range(2):
                                ps = pps.tile([128, 512], F32, tag="whps")
                                for fs in range(FS):
                                    nc.tensor.matmul(
                                        ps[:],
                                        vf[:, fs, kh * 128:(kh + 1) * 128],
                                        utg[:, fs, mch * 512:(mch + 1) * 512],
                                        start=(fs == 0), stop=(fs == FS - 1))
                                nc.scalar.copy(
                                    wht[:, kh,
                                        g * H + mch * 512:g * H + (mch + 1) * 512],
                                    ps[:])

            # ---------- phase 0c: cast WI^T to bf16 resident ----------
            with tc.tile_pool(name="p0c", bufs=2) as pc:
                for ic in range(ISUB):
                    wchunk = pc.tile([128, NG * H], F32, tag="wchunk")
                    nc.sync.dma_start(wchunk[:], wit_in[:, ic, :])
                    nc.scalar.copy(wit[:, ic, :], wchunk[:])

            # ---------- phase 0d: cast x to bf16 in DRAM ----------
            with tc.tile_pool(name="p0d", bufs=3) as pd:
                nrows = T * BL  # 2048
                for r in range(nrows // 128):
                    xc = pd.tile([128, I], F32, tag="xc")
                    nc.sync.dma_start(xc[:], x_in[r * 128:(r + 1) * 128, :])
                    xb = pd.tile([128, I], BF, tag="xb")
                    nc.vector.tensor_copy(xb[:], xc[:])
                    nc.sync.dma_start(xbf_d[r * 128:(r + 1) * 128, :], xb[:])

            # ---------- copy h0/c0 rows of the output ----------
            with tc.tile_pool(name="p0e", bufs=1) as pe:
                h0_sb = pe.tile([128, KSUB, BL], F32)
                nc.sync.dma_start(h0_sb[:], h0_in[:, :].rearrange(
                    "p (k b) -> p k b", k=KSUB))
                c0_sb = pe.tile([128, KSUB, BL], F32)
                nc.sync.dma_start(c0_sb[:], c0_in[:, :].rearrange(
                    "p (k b) -> p k b", k=KSUB))
                nc.sync.dma_start(
                    out[0, 0].rearrange("(k p) b -> p k b", p=128), h0_sb[:])
                nc.sync.dma_start(
                    out[1, 0].rearrange("(k p) b -> p k b", p=128), c0_sb[:])

                # ---------- phase 1: the scan ----------
                with tc.tile_pool(name="xt", bufs=3 * ISUB) as xt_pool, \
                     tc.tile_pool(name="act", bufs=4) as act, \
                     tc.tile_pool(name="hc", bufs=3) as hc, \
                     tc.tile_pool(name="zps", bufs=2, space="PSUM") as zpool:

                    hprev = hc.tile([128, KSUB, BL], BF, tag="hb")
                    nc.vector.tensor_copy(hprev[:], h0_sb[:])
                    c_cur = hc.tile([128, KSUB * BL], F32, tag="c")
                    nc.vector.tensor_copy(
                        c_cur[:], c0_sb[:].rearrange("p k b -> p (k b)"))

                    SIGM = 3 * (MT // 4)  # m-tiles through sigmoid = 24

                    for blk in range(NBLK):
                        # transpose this block's x rows: (TBLK*BL, I) -> 8 x (128, 64)
                        xts = []
                        for ic in range(ISUB):
                            xt = xt_pool.tile([128, TBLK * BL], BF, tag=f"xt{ic}")
                            nc.sync.dma_start_transpose(
                                xt[:],
                                xbf_d[blk * TBLK * BL:(blk + 1) * TBLK * BL,
                                      ic * 128:(ic + 1) * 128])
                            xts.append(xt)

                        zps = zpool.tile([128, MT, TBLK, BL], F32, tag="z")
                        # seed each PSUM bank with one whole-bank zero matmul:
                        # start=True clears has_written for the WHOLE bank, so
                        # exactly one start per bank; everything after
                        # accumulates (start=False) and is order-independent.
                        for bank in range(MT * TBLK * BL // 512):
                            nc.tensor.matmul(
                                zps[:, bank * 8:(bank + 1) * 8].rearrange(
                                    "p m t b -> p (m t b)"),
                                wht[:, 0, 0:128], zero_sb[:],
                                start=True, stop=False)
                        # xp for the whole block, batched: N = 64 per matmul
                        for m in range(MT):
                            zf = zps[:, m].rearrange("p a b -> p (a b)")
                            for ic in range(ISUB):
                                nc.tensor.matmul(
                                    zf, wit[:, ic, m * 128:(m + 1) * 128],
                                    xts[ic][:],
                                    start=False, stop=False)

                        for trel in range(TBLK):
                            t = blk * TBLK + trel
                            # recurrent part: accumulate onto xp in PSUM
                            for m in range(MT):
                                for kt in range(KSUB):
                                    nc.tensor.matmul(
                                        zps[:, m, trel, :],
                                        wht[:, kt, m * 128:(m + 1) * 128],
                                        hprev[:, kt, :],
                                        start=False,
                                        stop=(trel == TBLK - 1 and kt == KSUB - 1
                                              and m % 8 == 7))
                            # activations
                            sig = act.tile([128, SIGM * BL], F32, tag="sig")
                            nc.scalar.activation(
                                sig[:].rearrange("p (m b) -> p m b", m=SIGM),
                                zps[:, 0:SIGM, trel, :], AF.Sigmoid)
                            tg = act.tile([128, KSUB * BL], F32, tag="tg")
                            nc.scalar.activation(
                                tg[:].rearrange("p (m b) -> p m b", m=KSUB),
                                zps[:, SIGM:MT, trel, :], AF.Tanh)
                            ig = act.tile([128, KSUB * BL], F32, tag="ig")
                            nc.vector.tensor_mul(
                                ig[:], sig[:, :KSUB * BL], tg[:])
                            cnew = hc.tile([128, KSUB * BL], F32, tag="c")
                            nc.vector.tensor_mul(
                                cnew[:], sig[:, KSUB * BL:2 * KSUB * BL], c_cur[:])
                            nc.vector.tensor_add(cnew[:], cnew[:], ig[:])
                            tcn = act.tile([128, KSUB * BL], F32, tag="tc")
                            nc.scalar.activation(tcn[:], cnew[:], AF.Tanh)
                            hnew = act.tile([128, KSUB * BL], F32, tag="hnew")
                            nc.vector.tensor_mul(
                                hnew[:], sig[:, 2 * KSUB * BL:3 * KSUB * BL], tcn[:])
                            hb = hc.tile([128, KSUB, BL], BF, tag="hb")
                            nc.vector.tensor_copy(
                                hb[:], hnew[:].rearrange("p (k b) -> p k b", k=KSUB))
                            # write outputs
                            nc.sync.dma_start(
                                out[0, t + 1].rearrange("(k p) b -> p k b", p=128),
                                hnew[:].rearrange("p (k b) -> p k b", k=KSUB))
                            nc.sync.dma_start(
                                out[1, t + 1].rearrange("(k p) b -> p k b", p=128),
                                cnew[:].rearrange("p (k b) -> p k b", k=KSUB))
                            hprev = hb
                            c_cur = cnew
    nc.compile()
    return nc


def _prep_in_maps(tnsr_input, h0, c0, tnsr_cond, wei_F, wei_U, wei_V, wei_WI):
    perm = GATE_PERM
    f32 = np.float32
    # weights: replicated, host-side layout rearranges only
    wFp = np.ascontiguousarray(
        wei_F.reshape(F // 128, 128, C).transpose(1, 0, 2)).astype(f32)
    condb = np.ascontiguousarray(
        np.broadcast_to(tnsr_cond[None, :], (128, C))).astype(f32)
    # Vp[p, fs, g, h] = V[g, fs*128+p, h]
    Vp = np.ascontiguousarray(
        wei_V[perm].reshape(NG, F // 128, 128, H).transpose(2, 1, 0, 3)).astype(f32)
    # UTp[p, fs, g, h] = U[g, h, fs*128+p]
    UTp = np.ascontiguousarray(
        wei_U[perm].reshape(NG, H, F // 128, 128).transpose(3, 2, 0, 1)).astype(f32)
    # WITp[p, isub, g*H + zh] = WI[g, zh, isub*128+p]
    WITp = np.ascontiguousarray(
        wei_WI[perm].reshape(NG, H, ISUB, 128).transpose(3, 2, 0, 1)
        .reshape(128, ISUB, NG * H)).astype(f32)

    in_maps = []
    for cix in range(N_CORES):
        bsl = slice(cix * BL, (cix + 1) * BL)
        xs = np.ascontiguousarray(
            tnsr_input[:, bsl, :].reshape(T * BL, I)).astype(f32)
        h0c = np.ascontiguousarray(
            h0[:, bsl].reshape(KSUB, 128, BL).transpose(1, 0, 2)
            .reshape(128, KSUB * BL)).astype(f32)
        c0c = np.ascontiguousarray(
            c0[:, bsl].reshape(KSUB, 128, BL).transpose(1, 0, 2)
            .reshape(128, KSUB * BL)).astype(f32)
        in_maps.append({
            "x": xs, "h0c": h0c, "c0c": c0c, "wFp": wFp, "condb": condb,
            "Vp": Vp, "UTp": UTp, "WITp": WITp,
        })
    return in_maps


def kernel(**inputs):
    global _CACHED
    from concourse.bass_utils import run_bass_kernel_spmd

    if _CACHED is None:
        _CACHED = _build_program()
    nc = _CACHED

    in_maps = _prep_in_maps(
        np.asarray(inputs["tnsr_input"]), np.asarray(inputs["h0"]),
        np.asarray(inputs["c0"]), np.asarray(inputs["tnsr_cond"]),
        np.asarray(inputs["wei_F"]), np.asarray(inputs["wei_U"]),
        np.asarray(inputs["wei_V"]), np.asarray(inputs["wei_WI"]))

    res = run_bass_kernel_spmd(nc, in_maps, list(range(N_CORES)))
    full = np.empty((2, T + 1, H, B), dtype=np.float32)
    for cix in range(N_CORES):
        full[:, :, :, cix * BL:(cix + 1) * BL] = res.results[cix]["out"]
    return full


# revision 11
# speedup vs baseline: 1657.2420x; 1657.2420x over previous
"""Cond-LSTM Trainium2 kernel (nn_Cond_LSTM): batch-sharded SPMD over 8 NeuronCores.

Reference computation:
    f   = wei_F @ tnsr_cond                              (F,)
    WH  = einsum('ghf,f,gfk->ghk', wei_U, f, wei_V)      (4,H,H)
    xp  = einsum('ghi,tbi->tghb', wei_WI, tnsr_input)    (T,4,H,B)
    LSTM scan over T with z_t = xp_t + WH @ h_{t-1}
    out = stack([hs, cs])                                (2,T+1,H,B)

Sharding: data-parallel over batch (B=64 -> 8 per core); weights replicated.
All arithmetic runs on-device (bf16 matmuls, fp32 accumulation/activations).
Host does only slicing / layout rearranges / gather.
"""

import numpy as np

T, B, I, H, F, C = 256, 64, 1024, 1024, 512, 512
NG = 4            # gates
N_CORES = 8
BL = B // N_CORES  # local batch = 8
TBLK = 8           # recurrence steps per x-transpose/xp block
NBLK = T // TBLK
KSUB = H // 128    # 8 k-tiles over hidden dim
ISUB = I // 128    # 8 k-tiles over input dim
MT = NG * H // 128  # 32 m-tiles over z rows
# gate order used on-device: [i, f, o, g]; reference order is [i, f, g, o]
GATE_PERM = [0, 1, 3, 2]

_CACHED = None


def _build_program(timing=False):
    import concourse.bacc as bacc
    import concourse.mybir as mybir
    import concourse.tile as tile

    F32 = mybir.dt.float32
    BF = mybir.dt.bfloat16
    AF = mybir.ActivationFunctionType

    nc = bacc.Bacc("TRN2", target_bir_lowering=False, debug=False,
                   num_devices=N_CORES)

    # ---- per-core inputs (host-rearranged layouts, all fp32) ----
    x_in = nc.dram_tensor("x", [T * BL, I], F32, kind="ExternalInput")
    h0_in = nc.dram_tensor("h0c", [128, KSUB * BL], F32, kind="ExternalInput")
    c0_in = nc.dram_tensor("c0c", [128, KSUB * BL], F32, kind="ExternalInput")
    wf_in = nc.dram_tensor("wFp", [128, F // 128, C], F32, kind="ExternalInput")
    cond_in = nc.dram_tensor("condb", [128, C], F32, kind="ExternalInput")
    v_in = nc.dram_tensor("Vp", [128, F // 128, NG, H], F32, kind="ExternalInput")
    ut_in = nc.dram_tensor("UTp", [128, F // 128, NG, H], F32, kind="ExternalInput")
    wit_in = nc.dram_tensor("WITp", [128, ISUB, NG * H], F32, kind="ExternalInput")
    if timing:
        out = nc.dram_tensor("out", [2, T + 1, H, BL], F32)
        tok = nc.dram_tensor("tok", [128, KSUB * BL], F32, kind="ExternalOutput")
    else:
        out = nc.dram_tensor("out", [2, T + 1, H, BL], F32, kind="ExternalOutput")
        tok = None
    xbf_d = nc.dram_tensor("xbf_scratch", [T * BL, I], BF)

    FS = F // 128  # 4

    with tile.TileContext(nc) as tc:
        # ---------- resident weights ----------
        with tc.tile_pool(name="resident", bufs=1) as res:
            wht = res.tile([128, KSUB, NG * H], BF)   # lhsT for WH part (8 MB)
            wit = res.tile([128, ISUB, NG * H], BF)   # lhsT for xp part (8 MB)
            zero_sb = res.tile([128, 512], BF)
            nc.vector.memset(zero_sb[:], 0.0)

            # ---------- phase 0a: f = rowsum(wFp * cond) ----------
            with tc.tile_pool(name="p0a", bufs=1) as p0:
                wf_sb = p0.tile([128, FS, C], F32)
                nc.sync.dma_start(wf_sb[:], wf_in[:, :, :])
                cond_sb = p0.tile([128, C], F32)
                nc.sync.dma_start(cond_sb[:], cond_in[:, :])
                prod = p0.tile([128, FS, C], F32)
                nc.vector.tensor_tensor(
                    prod[:], wf_sb[:],
                    cond_sb[:, None, :].to_broadcast((128, FS, C)),
                    mybir.AluOpType.mult)
                fvec = p0.tile([128, FS], F32)
                nc.vector.reduce_sum(fvec[:], prod[:], axis=mybir.AxisListType.X)

                # ---------- phase 0b: WHT[g] = Vf[g]^T-contracted with UT[g] ----
                with tc.tile_pool(name="p0b", bufs=1) as pb, \
                     tc.tile_pool(name="p0ps", bufs=4, space="PSUM") as pps:
                    for g in range(NG):
                        utg_f = pb.tile([128, FS, H], F32, tag="utgf")
                        nc.sync.dma_start(utg_f[:], ut_in[:, :, g, :])
                        utg = pb.tile([128, FS, H], BF, tag="utg")
                        nc.scalar.copy(utg[:], utg_f[:])
                        vg_f = pb.tile([128, FS, H], F32, tag="vgf")
                        nc.sync.dma_start(vg_f[:], v_in[:, :, g, :])
                        vf = pb.tile([128, FS, H], BF, tag="vf")
                        nc.vector.tensor_tensor(
                            vf[:], vg_f[:],
                            fvec[:, :, None].to_broadcast((128, FS, H)),
                            mybir.AluOpType.mult)
                        for kh in range(KSUB):
                            for mch in range(2):
                                ps = pps.tile([128, 512], F32, tag="whps")
                                for fs in range(FS):
                                    nc.tensor.matmul(
                                        ps[:],
                                        vf[:, fs, kh * 128:(kh + 1) * 128],
                                        utg[:, fs, mch * 512:(mch + 1) * 512],
                                        start=(fs == 0), stop=(fs == FS - 1))
                                nc.scalar.copy(
                                    wht[:, kh,
                                        g * H + mch * 512:g * H + (mch + 1) * 512],
                                    ps[:])

            # ---------- phase 0c: cast WI^T to bf16 resident ----------
            with tc.tile_pool(name="p0c", bufs=2) as pc:
                for ic in range(ISUB):
                    wchunk = pc.tile([128, NG * H], F32, tag="wchunk")
                    nc.sync.dma_start(wchunk[:], wit_in[:, ic, :])
                    nc.scalar.copy(wit[:, ic, :], wchunk[:])

            # ---------- phase 0d: cast x to bf16 in DRAM ----------
            with tc.tile_pool(name="p0d", bufs=3) as pd:
                nrows = T * BL  # 2048
                for r in range(nrows // 128):
                    xc = pd.tile([128, I], F32, tag="xc")
                    nc.sync.dma_start(xc[:], x_in[r * 128:(r + 1) * 128, :])
                    xb = pd.tile([128, I], BF, tag="xb")
                    nc.vector.tensor_copy(xb[:], xc[:])
                    nc.sync.dma_start(xbf_d[r * 128:(r + 1) * 128, :], xb[:])

            # ---------- copy h0/c0 rows of the output ----------
            with tc.tile_pool(name="p0e", bufs=1) as pe:
                h0_sb = pe.tile([128, KSUB, BL], F32)
                nc.sync.dma_start(h0_sb[:], h0_in[:, :].rearrange(
                    "p (k b) -> p k b", k=KSUB))
                c0_sb = pe.tile([128, KSUB, BL], F32)
                nc.sync.dma_start(c0_sb[:], c0_in[:, :].rearrange(
                    "p (k b) -> p k b", k=KSUB))
                nc.sync.dma_start(
                    out[0, 0].rearrange("(k p) b -> p k b", p=128), h0_sb[:])
                nc.sync.dma_start(
                    out[1, 0].rearrange("(k p) b -> p k b", p=128), c0_sb[:])

                # ---------- phase 1: the scan ----------
                with tc.tile_pool(name="xt", bufs=3 * ISUB) as xt_pool, \
                     tc.tile_pool(name="act", bufs=4) as act, \
                     tc.tile_pool(name="hc", bufs=3) as hc, \
                     tc.tile_pool(name="zps", bufs=2, space="PSUM") as zpool:

                    hprev = hc.tile([128, KSUB, BL], BF, tag="hb")
                    nc.vector.tensor_copy(hprev[:], h0_sb[:])
                    c_cur = hc.tile([128, KSUB * BL], F32, tag="c")
                    nc.vector.tensor_copy(
                        c_cur[:], c0_sb[:].rearrange("p k b -> p (k b)"))

                    SIGM = 3 * (MT // 4)  # m-tiles through sigmoid = 24

                    for blk in range(NBLK):
                        # transpose this block's x rows: (TBLK*BL, I) -> 8 x (128, 64)
                        xts = []
                        for ic in range(ISUB):
                            xt = xt_pool.tile([128, TBLK * BL], BF, tag=f"xt{ic}")
                            nc.sync.dma_start_transpose(
                                xt[:],
                                xbf_d[blk * TBLK * BL:(blk + 1) * TBLK * BL,
                                      ic * 128:(ic + 1) * 128])
                            xts.append(xt)

                        zps = zpool.tile([128, MT, TBLK, BL], F32, tag="z")
                        # seed each PSUM bank with one whole-bank zero matmul:
                        # start=True clears has_written for the WHOLE bank, so
                        # exactly one start per bank; everything after
                        # accumulates (start=False) and is order-independent.
                        for bank in range(MT * TBLK * BL // 512):
                            nc.tensor.matmul(
                                zps[:, bank * 8:(bank + 1) * 8].rearrange(
                                    "p m t b -> p (m t b)"),
                                wht[:, 0, 0:128], zero_sb[:],
                                start=True, stop=False)
                        # xp for the whole block, batched: N = 64 per matmul
                        for m in range(MT):
                            zf = zps[:, m].rearrange("p a b -> p (a b)")
                            for ic in range(ISUB):
                                nc.tensor.matmul(
                                    zf, wit[:, ic, m * 128:(m + 1) * 128],
                                    xts[ic][:],
                                    start=False, stop=False)

                        for trel in range(TBLK):
                            t = blk * TBLK + trel
                            # recurrent part: accumulate onto xp in PSUM.
                            # g-gate tiles (m 24..31, their own PSUM bank) go
                            # first so the tanh overlaps the i/f/o matmuls.
                            for m in list(range(SIGM, MT)) + list(range(SIGM)):
                                for kt in range(KSUB):
                                    nc.tensor.matmul(
                                        zps[:, m, trel, :],
                                        wht[:, kt, m * 128:(m + 1) * 128],
                                        hprev[:, kt, :],
                                        start=False,
                                        stop=(trel == TBLK - 1 and kt == KSUB - 1
                                              and m % 8 == 7))
                            # activations (tanh first: its inputs finish first)
                            tg = act.tile([128, KSUB * BL], F32, tag="tg")
                            nc.scalar.activation(
                                tg[:].rearrange("p (m b) -> p m b", m=KSUB),
                                zps[:, SIGM:MT, trel, :], AF.Tanh)
                            sig = act.tile([128, SIGM * BL], F32, tag="sig")
                            nc.scalar.activation(
                                sig[:].rearrange("p (m b) -> p m b", m=SIGM),
                                zps[:, 0:SIGM, trel, :], AF.Sigmoid)
                            ig = act.tile([128, KSUB * BL], F32, tag="ig")
                            nc.vector.tensor_mul(
                                ig[:], sig[:, :KSUB * BL], tg[:])
                            cnew = hc.tile([128, KSUB * BL], F32, tag="c")
                            nc.vector.tensor_mul(
                                cnew[:], sig[:, KSUB * BL:2 * KSUB * BL], c_cur[:])
                            nc.vector.tensor_add(cnew[:], cnew[:], ig[:])
                            tcn = act.tile([128, KSUB * BL], F32, tag="tc")
                            nc.scalar.activation(tcn[:], cnew[:], AF.Tanh)
                            # bf16 h for the next step first (critical path),
                            # f32 copy for the output afterwards
                            hb = hc.tile([128, KSUB, BL], BF, tag="hb")
                            nc.vector.tensor_mul(
                                hb[:].rearrange("p k b -> p (k b)"),
                                sig[:, 2 * KSUB * BL:3 * KSUB * BL], tcn[:])
                            hnew = act.tile([128, KSUB * BL], F32, tag="hnew")
                            nc.vector.tensor_mul(
                                hnew[:], sig[:, 2 * KSUB * BL:3 * KSUB * BL], tcn[:])
                            # write outputs
                            nc.sync.dma_start(
                                out[0, t + 1].rearrange("(k p) b -> p k b", p=128),
                                hnew[:].rearrange("p (k b) -> p k b", k=KSUB))
                            nc.sync.dma_start(
                                out[1, t + 1].rearrange("(k p) b -> p k b", p=128),
                                cnew[:].rearrange("p (k b) -> p k b", k=KSUB))
                            hprev = hb
                            c_cur = cnew
                    if timing:
                        nc.sync.dma_start(tok[:, :], hnew[:])
    nc.compile()
    return nc


def _prep_in_maps(tnsr_input, h0, c0, tnsr_cond, wei_F, wei_U, wei_V, wei_WI):
    perm = GATE_PERM
    f32 = np.float32
    # weights: replicated, host-side layout rearranges only
    wFp = np.ascontiguousarray(
        wei_F.reshape(F // 128, 128, C).transpose(1, 0, 2)).astype(f32)
    condb = np.ascontiguousarray(
        np.broadcast_to(tnsr_cond[None, :], (128, C))).astype(f32)
    # Vp[p, fs, g, h] = V[g, fs*128+p, h]
    Vp = np.ascontiguousarray(
        wei_V[perm].reshape(NG, F // 128, 128, H).transpose(2, 1, 0, 3)).astype(f32)
    # UTp[p, fs, g, h] = U[g, h, fs*128+p]
    UTp = np.ascontiguousarray(
        wei_U[perm].reshape(NG, H, F // 128, 128).transpose(3, 2, 0, 1)).astype(f32)
    # WITp[p, isub, g*H + zh] = WI[g, zh, isub*128+p]
    WITp = np.ascontiguousarray(
        wei_WI[perm].reshape(NG, H, ISUB, 128).transpose(3, 2, 0, 1)
        .reshape(128, ISUB, NG * H)).astype(f32)

    in_maps = []
    for cix in range(N_CORES):
        bsl = slice(cix * BL, (cix + 1) * BL)
        xs = np.ascontiguousarray(
            tnsr_input[:, bsl, :].reshape(T * BL, I)).astype(f32)
        h0c = np.ascontiguousarray(
            h0[:, bsl].reshape(KSUB, 128, BL).transpose(1, 0, 2)
            .reshape(128, KSUB * BL)).astype(f32)
        c0c = np.ascontiguousarray(
            c0[:, bsl].reshape(KSUB, 128, BL).transpose(1, 0, 2)
            .reshape(128, KSUB * BL)).astype(f32)
        in_maps.append({
            "x": xs, "h0c": h0c, "c0c": c0c, "wFp": wFp, "condb": condb,
            "Vp": Vp, "UTp": UTp, "WITp": WITp,
        })
    return in_maps


def kernel(**inputs):
    global _CACHED
    from concourse.bass_utils import run_bass_kernel_spmd

    if _CACHED is None:
        _CACHED = _build_program()
    nc = _CACHED

    in_maps = _prep_in_maps(
        np.asarray(inputs["tnsr_input"]), np.asarray(inputs["h0"]),
        np.asarray(inputs["c0"]), np.asarray(inputs["tnsr_cond"]),
        np.asarray(inputs["wei_F"]), np.asarray(inputs["wei_U"]),
        np.asarray(inputs["wei_V"]), np.asarray(inputs["wei_WI"]))

    res = run_bass_kernel_spmd(nc, in_maps, list(range(N_CORES)))
    full = np.empty((2, T + 1, H, B), dtype=np.float32)
    for cix in range(N_CORES):
        full[:, :, :, cix * BL:(cix + 1) * BL] = res.results[cix]["out"]
    return full
